# revision 18
# baseline (speedup 1.0000x reference)
"""BiasAttention TRN2 kernel — q-sharded across 8 NeuronCores.

Each core owns a block of 128 queries and computes the full attention for
them (all 8 heads, all 1024 keys), including the z-bias projection, with no
collectives.  Host-side prep re-lays z out per core as [g, c, t, q] so the
bias-projection tiles arrive in SBUF matmul-ready (contract dim c on
partitions), and casts the matmul datapath to bf16 (TRN2 fp32 matmuls run
in split LOW/HIGH mode at ~4x the cost; accumulation stays fp32 in PSUM).
"""

import sys

if "/opt/trn_rl_repo" not in sys.path:
    sys.path.insert(0, "/opt/trn_rl_repo")

import ml_dtypes
import numpy as np

import concourse.bass as bass
import concourse.mybir as mybir
from concourse import bacc
from concourse.bass_utils import run_bass_kernel_spmd
from concourse.masks import make_identity
from concourse.tile import TileContext

P = 128          # partitions
H = 8            # heads
D = 32           # head dim
CQ = 256         # q channels
CKV = 256        # kv channels
BD = 128         # bias (z) channels
NQ = 1024        # total queries
NCORES = 8
NQC = NQ // NCORES   # queries per core = 128
SCALE = D ** (-0.5)

GK = 16          # z k-tiles per DMA group (shared host/device)
FP = mybir.dt.float32
BF = mybir.dt.bfloat16
NP_BF = ml_dtypes.bfloat16


def build_program(nk=1024, gk=GK):
    """Build the SPMD single-core program.  nk = number of keys, gk = z
    k-tiles per DMA group."""
    kc_n = nk // P            # k-chunks of 128
    ng = nk // gk             # z DMA groups
    add = mybir.AluOpType.add
    mult = mybir.AluOpType.mult

    nc = bacc.Bacc("TRN2", target_bir_lowering=False, debug=False,
                   num_devices=NCORES)

    # ---- I/O ----
    zT = nc.dram_tensor("zT", [ng, BD, gk, NQC], BF, kind="ExternalInput")
    xqT = nc.dram_tensor("xqT", [CQ, NQC], BF, kind="ExternalInput")
    xkvT = nc.dram_tensor("xkvT", [CKV, nk], BF, kind="ExternalInput")
    Wq = nc.dram_tensor("Wq", [CQ, H * D], BF, kind="ExternalInput")
    bq = nc.dram_tensor("bq", [H * D], FP, kind="ExternalInput")
    Wkv = nc.dram_tensor("Wkv", [CKV, 2 * H * D], BF, kind="ExternalInput")
    bkv = nc.dram_tensor("bkv", [2 * H * D], FP, kind="ExternalInput")
    Wb = nc.dram_tensor("Wb", [BD, H], BF, kind="ExternalInput")
    bb = nc.dram_tensor("bb", [H], FP, kind="ExternalInput")
    Wp = nc.dram_tensor("Wp", [H * D, CQ], FP, kind="ExternalInput")
    bp = nc.dram_tensor("bp", [CQ], FP, kind="ExternalInput")
    y = nc.dram_tensor("y", [NQC, CQ], FP, kind="ExternalOutput")

    with TileContext(nc) as tc:
        with (
            tc.tile_pool(name="const", bufs=1) as const,
            tc.tile_pool(name="zpool", bufs=14) as zpool,
            tc.tile_pool(name="epool", bufs=3) as epool,
            tc.tile_pool(name="atpool", bufs=3) as atpool,
            tc.tile_pool(name="proj_ps", bufs=3, space="PSUM") as proj_ps,
            tc.tile_pool(name="b_ps", bufs=2, space="PSUM") as b_psp,
            tc.tile_pool(name="t_ps", bufs=2, space="PSUM") as t_psp,
            tc.tile_pool(name="o_ps", bufs=1, space="PSUM") as o_psp,
        ):
            # ---- constants / weights to SBUF ----
            wq_sb = const.tile([P, 2, H * D], BF)
            nc.gpsimd.dma_start(wq_sb, Wq.rearrange("(o p) m -> p o m", p=P))
            wkv_sb = const.tile([P, 2, 2 * H * D], BF)
            nc.gpsimd.dma_start(wkv_sb, Wkv.rearrange("(o p) m -> p o m", p=P))
            wb_sb = const.tile([P, H], BF)
            nc.gpsimd.dma_start(wb_sb, Wb[:])
            wp_sb = const.tile([P, 2, CQ], FP)
            nc.gpsimd.dma_start(wp_sb, Wp.rearrange("(o p) m -> p o m", p=P))
            xqT_sb = const.tile([P, 2, NQC], BF)
            nc.gpsimd.dma_start(xqT_sb, xqT.rearrange("(o p) q -> p o q", p=P))
            xkvT_sb = const.tile([P, 2, nk], BF)
            nc.gpsimd.dma_start(xkvT_sb, xkvT.rearrange("(o p) k -> p o k", p=P))
            bq_sb = const.tile([P, 2], FP)
            nc.gpsimd.dma_start(bq_sb, bq.rearrange("(o p) -> p o", p=P))
            bkvK_sb = const.tile([P, 2], FP)
            nc.gpsimd.dma_start(bkvK_sb, bkv[0:H * D].rearrange("(o p) -> p o", p=P))
            bkvV_sb = const.tile([1, H * D], FP)
            nc.gpsimd.dma_start(bkvV_sb, bkv[None, H * D:2 * H * D])
            bp_sb = const.tile([1, CQ], FP)
            nc.gpsimd.dma_start(bp_sb, bp[None, :])
            bb_ap = bb[:]
            bb_sb = const.tile([P, H], FP)
            nc.gpsimd.dma_start(
                out=bb_sb,
                in_=bass.AP(tensor=bb_ap.tensor, offset=bb_ap.offset,
                            ap=[[0, P]] + list(bb_ap.ap)),
            )
            ident = const.tile([P, P], FP)
            make_identity(nc, ident)
            ident_bf = const.tile([P, P], BF)
            make_identity(nc, ident_bf)
            ones_row = const.tile([1, P], FP)
            nc.vector.memset(ones_row, 1.0)

            # V augmented with a ones column per head: [k, kc, h, D+1]
            vaug_sb = const.tile([P, kc_n, H, D + 1], BF)
            nc.vector.memset(vaug_sb, 1.0)


            # ---- projections (bf16 in, fp32 psum accumulate) ----
            # Q^T [(h d), q] with (x + bq) * SCALE folded in, stored bf16
            qT_sb = const.tile([P, 2, NQC], BF)
            for m in range(2):
                ps = proj_ps.tile([P, 512], FP, tag="proj")
                for c in range(2):
                    nc.tensor.matmul(ps[:, :NQC],
                                     lhsT=wq_sb[:, c, m * P:(m + 1) * P],
                                     rhs=xqT_sb[:, c, :],
                                     start=(c == 0), stop=(c == 1))
                nc.vector.tensor_scalar(qT_sb[:, m, :], ps[:, :NQC],
                                        bq_sb[:, m:m + 1], SCALE, add, mult)

            # K^T [(h d), k] with +bkv_K, stored bf16
            kT_sb = const.tile([P, 2, nk], BF)
            for m in range(2):
                for nh in range((nk + 511) // 512):
                    nn_ = min(512, nk - nh * 512)
                    ps = proj_ps.tile([P, 512], FP, tag="proj")
                    for c in range(2):
                        nc.tensor.matmul(ps[:, :nn_],
                                         lhsT=wkv_sb[:, c, m * P:(m + 1) * P],
                                         rhs=xkvT_sb[:, c, nh * 512:nh * 512 + nn_],
                                         start=(c == 0), stop=(c == 1))
                    nc.vector.tensor_scalar(kT_sb[:, m, nh * 512:nh * 512 + nn_],
                                            ps[:, :nn_], bkvK_sb[:, m:m + 1],
                                            None, add)


            # S[q, h, k] = SCALE * Q K^T + bb  (scale folded into Q already)
            s_sb = const.tile([P, H, nk], FP)
            for h in range(H):
                r0 = (h % 4) * 32
                for nh in range((nk + 511) // 512):
                    nn_ = min(512, nk - nh * 512)
                    ps = proj_ps.tile([P, 512], FP, tag="proj", name="qk_ps")
                    nc.tensor.matmul(ps[:, :nn_],
                                     lhsT=qT_sb[r0:r0 + 32, h // 4, :],
                                     rhs=kT_sb[r0:r0 + 32, h // 4,
                                               nh * 512:nh * 512 + nn_],
                                     start=True, stop=True,
                                     tile_position=(r0, 0))
                    # Identity-with-bias folds bb[h] into S; alternate the
                    # copy between ACT and DVE to halve the serial chain.
                    if (h * 2 + nh) % 2 == 0:
                        nc.scalar.activation(
                            s_sb[:, h, nh * 512:nh * 512 + nn_], ps[:, :nn_],
                            mybir.ActivationFunctionType.Identity,
                            bias=bb_sb[:, h:h + 1])
                    else:
                        nc.vector.tensor_scalar(
                            s_sb[:, h, nh * 512:nh * 512 + nn_], ps[:, :nn_],
                            bb_sb[:, h:h + 1], None, add)



            # V [k, (h d)] + bkv_V, written into vaug (ones col preserved);
            # emitted after QK so the S chain starts first.
            for kc in range(kc_n):
                ps = proj_ps.tile([P, 512], FP, tag="proj", name="v_ps")
                for c in range(2):
                    nc.tensor.matmul(ps[:, :H * D],
                                     lhsT=xkvT_sb[:, c, kc * P:(kc + 1) * P],
                                     rhs=wkv_sb[:, c, H * D:2 * H * D],
                                     start=(c == 0), stop=False)
                nc.tensor.matmul(ps[:, :H * D], lhsT=ones_row,
                                 rhs=bkvV_sb, start=False, stop=True)
                nc.scalar.activation(
                    vaug_sb[:, kc, :, 0:D],
                    ps[:, :H * D].rearrange("p (h d) -> p h d", h=H),
                    mybir.ActivationFunctionType.Copy)
            # ---- main loop over k-chunks ----
            o_ps = o_psp.tile([P, H * (D + 1)], FP)   # [q, h*(D+1)]
            gpc = P // gk if gk < P else 1      # groups per k-chunk
            tpg = min(gk, P)                    # k-tiles per group
            HKT = 64                      # k-tiles per half-chunk (1 psum bank)
            gph = HKT // tpg              # z DMA groups per half-chunk
            for kc in range(kc_n):
                x_sb = epool.tile([P, H, P], BF, tag="x")
                for hf in range(2):
                    # z-bias matmuls accumulate into b_ps [q, kt*H + h]
                    b_ps = b_psp.tile([P, HKT * H], FP, tag="b")
                    for g in range(gph):
                        gidx = kc * gpc + hf * gph + g
                        z_sb = zpool.tile([P, tpg, NQC], BF, tag="z")
                        nc.sync.dma_start(z_sb, zT[gidx])
                        for t in range(tpg):
                            kt = g * tpg + t
                            nc.tensor.matmul(b_ps[:, kt * H:(kt + 1) * H],
                                             lhsT=z_sb[:, t, :], rhs=wb_sb,
                                             start=(kt == 0),
                                             stop=(kt == HKT - 1))
                    # batched add + exp for this half-chunk (all 8 heads)
                    e_sb = epool.tile([P, H, HKT], FP, tag="e")
                    nc.vector.tensor_tensor(
                        e_sb,
                        s_sb[:, :, kc * P + hf * HKT:kc * P + (hf + 1) * HKT],
                        b_ps.rearrange("p (kt h) -> p h kt", h=H), add)
                    nc.scalar.activation(x_sb[:, :, hf * HKT:(hf + 1) * HKT],
                                         e_sb,
                                         mybir.ActivationFunctionType.Exp)
                for hg in range(2):          # head groups of 4
                    t_ps = t_psp.tile([P, 4, P], BF, tag="t")
                    for hl in range(4):
                        nc.tensor.transpose(t_ps[:, hl, :],
                                            x_sb[:, hg * 4 + hl, :], ident_bf)
                    at_sb = atpool.tile([P, 4, P], BF, tag="at")
                    nc.vector.tensor_copy(at_sb, t_ps)
                    for hl in range(4):
                        h = hg * 4 + hl
                        # o_ps lives in one bank: open the accumulation group
                        # on the first matmul only, close on the last.
                        nc.tensor.matmul(
                            o_ps[:, h * (D + 1):(h + 1) * (D + 1)],
                            lhsT=at_sb[:, hl, :], rhs=vaug_sb[:, kc, h, :],
                            start=(kc == 0 and h == 0),
                            stop=(kc == kc_n - 1 and h == H - 1))

            # ---- epilogue: normalize, transpose, output projection ----
            recip_sb = const.tile([P, H], FP)
            for h in range(H):
                nc.vector.reciprocal(recip_sb[:, h:h + 1],
                                     o_ps[:, h * (D + 1) + D:h * (D + 1) + D + 1])
            o_sb = const.tile([P, 2, P], FP)     # [q, half, (h d)%128]
            for h in range(H):
                nc.vector.tensor_scalar(
                    o_sb[:, h // 4, (h % 4) * 32:(h % 4) * 32 + 32],
                    o_ps[:, h * (D + 1):h * (D + 1) + D],
                    recip_sb[:, h:h + 1], None, mult)
            oT_sb = const.tile([P, 2, P], FP)
            for m in range(2):
                t_full = proj_ps.tile([P, 512], FP, tag="proj", name="t_full")
                t_ps = t_full[:, :P]
                nc.tensor.transpose(t_ps, o_sb[:, m, :], ident)
                nc.vector.tensor_copy(oT_sb[:, m, :], t_ps)
            ps = proj_ps.tile([P, 512], FP, tag="proj")
            for m in range(2):
                nc.tensor.matmul(ps[:, :CQ], lhsT=oT_sb[:, m, :],
                                 rhs=wp_sb[:, m, :], start=(m == 0), stop=False)
            nc.tensor.matmul(ps[:, :CQ], lhsT=ones_row, rhs=bp_sb,
                             start=False, stop=True)
            y_sb = const.tile([P, CQ], FP)
            nc.vector.tensor_copy(y_sb, ps[:, :CQ])
            nc.sync.dma_start(y[:], y_sb)

    nc.compile()
    return nc


def prep_inputs(x_q, x_kv, z, Wq, bq, Wkv, bkv, Wb, bb, Wp, bp,
                nk=1024, gk=GK):
    """Host-side shard prep.  Returns in_maps for the 8 cores."""
    ng = nk // gk
    xkvT = np.ascontiguousarray(x_kv[0].T).astype(NP_BF)     # [CKV, nk]
    shared = dict(xkvT=xkvT,
                  Wq=np.ascontiguousarray(Wq).astype(NP_BF),
                  bq=np.ascontiguousarray(bq, dtype=np.float32),
                  Wkv=np.ascontiguousarray(Wkv).astype(NP_BF),
                  bkv=np.ascontiguousarray(bkv, dtype=np.float32),
                  Wb=np.ascontiguousarray(Wb).astype(NP_BF),
                  bb=np.ascontiguousarray(bb, dtype=np.float32),
                  Wp=np.ascontiguousarray(Wp, dtype=np.float32),
                  bp=np.ascontiguousarray(bp, dtype=np.float32))
    in_maps = []
    for i in range(NCORES):
        qs = i * NQC
        zi = z[0, qs:qs + NQC]                           # [q, k, c]
        # -> [g, c, t, q] with k = g*gk + t
        zi = zi.reshape(NQC, ng, gk, BD).transpose(1, 3, 2, 0)
        in_maps.append(dict(
            zT=np.ascontiguousarray(zi).astype(NP_BF),
            xqT=np.ascontiguousarray(x_q[0, qs:qs + NQC].T).astype(NP_BF),
            **shared,
        ))
    return in_maps


_NC_CACHE = {}


def kernel(x_q, x_kv, z, Wq, bq, Wkv, bkv, Wb, bb, Wp, bp):
    key = "full"
    if key not in _NC_CACHE:
        _NC_CACHE[key] = build_program()
    nc = _NC_CACHE[key]
    in_maps = prep_inputs(x_q, x_kv, z, Wq, bq, Wkv, bkv, Wb, bb, Wp, bp)
    res = run_bass_kernel_spmd(nc, in_maps, list(range(NCORES)))
    out = np.empty((1, NQ, CQ), dtype=np.float32)
    for i in range(NCORES):
        out[0, i * NQC:(i + 1) * NQC, :] = res.results[i]["y"]
    return out


# revision 19
# speedup vs baseline: 1.0190x; 1.0190x over previous
"""BiasAttention TRN2 kernel — q-sharded across 8 NeuronCores.

Each core owns a block of 128 queries and computes the full attention for
them (all 8 heads, all 1024 keys), including the z-bias projection, with no
collectives.  Host-side prep re-lays z out per core as [g, c, t, q] so the
bias-projection tiles arrive in SBUF matmul-ready (contract dim c on
partitions), and casts the matmul datapath to bf16 (TRN2 fp32 matmuls run
in split LOW/HIGH mode at ~4x the cost; accumulation stays fp32 in PSUM).
"""

import sys

if "/opt/trn_rl_repo" not in sys.path:
    sys.path.insert(0, "/opt/trn_rl_repo")

import ml_dtypes
import numpy as np

import concourse.bass as bass
import concourse.mybir as mybir
from concourse import bacc
from concourse.bass_utils import run_bass_kernel_spmd
from concourse.masks import make_identity
from concourse.tile import TileContext

P = 128          # partitions
H = 8            # heads
D = 32           # head dim
CQ = 256         # q channels
CKV = 256        # kv channels
BD = 128         # bias (z) channels
NQ = 1024        # total queries
NCORES = 8
NQC = NQ // NCORES   # queries per core = 128
SCALE = D ** (-0.5)

GK = 16          # z k-tiles per DMA group (shared host/device)
FP = mybir.dt.float32
BF = mybir.dt.bfloat16
NP_BF = ml_dtypes.bfloat16


def build_program(nk=1024, gk=GK):
    """Build the SPMD single-core program.  nk = number of keys, gk = z
    k-tiles per DMA group."""
    kc_n = nk // P            # k-chunks of 128
    ng = nk // gk             # z DMA groups
    add = mybir.AluOpType.add
    mult = mybir.AluOpType.mult

    nc = bacc.Bacc("TRN2", target_bir_lowering=False, debug=False,
                   num_devices=NCORES)

    # ---- I/O ----
    zT = nc.dram_tensor("zT", [ng, BD, gk, NQC], BF, kind="ExternalInput")
    xqT = nc.dram_tensor("xqT", [CQ, NQC], BF, kind="ExternalInput")
    xkvT = nc.dram_tensor("xkvT", [CKV, nk], BF, kind="ExternalInput")
    Wq = nc.dram_tensor("Wq", [CQ, H * D], BF, kind="ExternalInput")
    bq = nc.dram_tensor("bq", [H * D], FP, kind="ExternalInput")
    Wkv = nc.dram_tensor("Wkv", [CKV, 2 * H * D], BF, kind="ExternalInput")
    bkv = nc.dram_tensor("bkv", [2 * H * D], FP, kind="ExternalInput")
    Wb = nc.dram_tensor("Wb", [BD, H], BF, kind="ExternalInput")
    bb = nc.dram_tensor("bb", [H], FP, kind="ExternalInput")
    Wp = nc.dram_tensor("Wp", [H * D, CQ], FP, kind="ExternalInput")
    bp = nc.dram_tensor("bp", [CQ], FP, kind="ExternalInput")
    y = nc.dram_tensor("y", [NQC, CQ], FP, kind="ExternalOutput")

    with TileContext(nc) as tc:
        with (
            tc.tile_pool(name="const", bufs=1) as const,
            tc.tile_pool(name="zpool", bufs=14) as zpool,
            tc.tile_pool(name="epool", bufs=3) as epool,
            tc.tile_pool(name="atpool", bufs=3) as atpool,
            tc.tile_pool(name="proj_ps", bufs=3, space="PSUM") as proj_ps,
            tc.tile_pool(name="b_ps", bufs=2, space="PSUM") as b_psp,
            tc.tile_pool(name="t_ps", bufs=2, space="PSUM") as t_psp,
            tc.tile_pool(name="o_ps", bufs=1, space="PSUM") as o_psp,
        ):
            # ---- constants / weights to SBUF ----
            wq_sb = const.tile([P, 2, H * D], BF)
            nc.sync.dma_start(wq_sb, Wq.rearrange("(o p) m -> p o m", p=P))
            wkv_sb = const.tile([P, 2, 2 * H * D], BF)
            nc.sync.dma_start(wkv_sb, Wkv.rearrange("(o p) m -> p o m", p=P))
            wb_sb = const.tile([P, H], BF)
            nc.sync.dma_start(wb_sb, Wb[:])
            wp_sb = const.tile([P, 2, CQ], FP)
            nc.sync.dma_start(wp_sb, Wp.rearrange("(o p) m -> p o m", p=P))
            xqT_sb = const.tile([P, 2, NQC], BF)
            nc.sync.dma_start(xqT_sb, xqT.rearrange("(o p) q -> p o q", p=P))
            xkvT_sb = const.tile([P, 2, nk], BF)
            nc.sync.dma_start(xkvT_sb, xkvT.rearrange("(o p) k -> p o k", p=P))
            bq_sb = const.tile([P, 2], FP)
            nc.sync.dma_start(bq_sb, bq.rearrange("(o p) -> p o", p=P))
            bkvK_sb = const.tile([P, 2], FP)
            nc.sync.dma_start(bkvK_sb, bkv[0:H * D].rearrange("(o p) -> p o", p=P))
            bkvV_sb = const.tile([1, H * D], FP)
            nc.sync.dma_start(bkvV_sb, bkv[None, H * D:2 * H * D])
            bp_sb = const.tile([1, CQ], FP)
            nc.sync.dma_start(bp_sb, bp[None, :])
            bb_ap = bb[:]
            bb_sb = const.tile([P, H], FP)
            nc.gpsimd.dma_start(
                out=bb_sb,
                in_=bass.AP(tensor=bb_ap.tensor, offset=bb_ap.offset,
                            ap=[[0, P]] + list(bb_ap.ap)),
            )
            ident = const.tile([P, P], FP)
            make_identity(nc, ident)
            ident_bf = const.tile([P, P], BF)
            make_identity(nc, ident_bf)
            ones_row = const.tile([1, P], FP)
            nc.vector.memset(ones_row, 1.0)

            # V augmented with a ones column per head: [k, kc, h, D+1]
            vaug_sb = const.tile([P, kc_n, H, D + 1], BF)
            nc.vector.memset(vaug_sb, 1.0)


            # ---- projections (bf16 in, fp32 psum accumulate) ----
            # Q^T [(h d), q] with (x + bq) * SCALE folded in, stored bf16
            qT_sb = const.tile([P, 2, NQC], BF)
            for m in range(2):
                ps = proj_ps.tile([P, 512], FP, tag="proj")
                for c in range(2):
                    nc.tensor.matmul(ps[:, :NQC],
                                     lhsT=wq_sb[:, c, m * P:(m + 1) * P],
                                     rhs=xqT_sb[:, c, :],
                                     start=(c == 0), stop=(c == 1))
                nc.vector.tensor_scalar(qT_sb[:, m, :], ps[:, :NQC],
                                        bq_sb[:, m:m + 1], SCALE, add, mult)

            # K^T [(h d), k] with +bkv_K, stored bf16
            kT_sb = const.tile([P, 2, nk], BF)
            for m in range(2):
                for nh in range((nk + 511) // 512):
                    nn_ = min(512, nk - nh * 512)
                    ps = proj_ps.tile([P, 512], FP, tag="proj")
                    for c in range(2):
                        nc.tensor.matmul(ps[:, :nn_],
                                         lhsT=wkv_sb[:, c, m * P:(m + 1) * P],
                                         rhs=xkvT_sb[:, c, nh * 512:nh * 512 + nn_],
                                         start=(c == 0), stop=(c == 1))
                    nc.vector.tensor_scalar(kT_sb[:, m, nh * 512:nh * 512 + nn_],
                                            ps[:, :nn_], bkvK_sb[:, m:m + 1],
                                            None, add)


            # S[q, h, k] = SCALE * Q K^T + bb  (scale folded into Q already)
            s_sb = const.tile([P, H, nk], FP)
            for h in range(H):
                r0 = (h % 4) * 32
                for nh in range((nk + 511) // 512):
                    nn_ = min(512, nk - nh * 512)
                    ps = proj_ps.tile([P, 512], FP, tag="proj", name="qk_ps")
                    nc.tensor.matmul(ps[:, :nn_],
                                     lhsT=qT_sb[r0:r0 + 32, h // 4, :],
                                     rhs=kT_sb[r0:r0 + 32, h // 4,
                                               nh * 512:nh * 512 + nn_],
                                     start=True, stop=True,
                                     tile_position=(r0, 0))
                    # Identity-with-bias folds bb[h] into S; alternate the
                    # copy between ACT and DVE to halve the serial chain.
                    if (h * 2 + nh) % 2 == 0:
                        nc.scalar.activation(
                            s_sb[:, h, nh * 512:nh * 512 + nn_], ps[:, :nn_],
                            mybir.ActivationFunctionType.Identity,
                            bias=bb_sb[:, h:h + 1])
                    else:
                        nc.vector.tensor_scalar(
                            s_sb[:, h, nh * 512:nh * 512 + nn_], ps[:, :nn_],
                            bb_sb[:, h:h + 1], None, add)



            # V [k, (h d)] + bkv_V, written into vaug (ones col preserved);
            # emitted after QK so the S chain starts first.
            for kc in range(kc_n):
                ps = proj_ps.tile([P, 512], FP, tag="proj", name="v_ps")
                for c in range(2):
                    nc.tensor.matmul(ps[:, :H * D],
                                     lhsT=xkvT_sb[:, c, kc * P:(kc + 1) * P],
                                     rhs=wkv_sb[:, c, H * D:2 * H * D],
                                     start=(c == 0), stop=False)
                nc.tensor.matmul(ps[:, :H * D], lhsT=ones_row,
                                 rhs=bkvV_sb, start=False, stop=True)
                nc.scalar.activation(
                    vaug_sb[:, kc, :, 0:D],
                    ps[:, :H * D].rearrange("p (h d) -> p h d", h=H),
                    mybir.ActivationFunctionType.Copy)
            # ---- main loop over k-chunks ----
            o_ps = o_psp.tile([P, H * (D + 1)], FP)   # [q, h*(D+1)]
            gpc = P // gk if gk < P else 1      # groups per k-chunk
            tpg = min(gk, P)                    # k-tiles per group
            HKT = 64                      # k-tiles per half-chunk (1 psum bank)
            gph = HKT // tpg              # z DMA groups per half-chunk
            for kc in range(kc_n):
                x_sb = epool.tile([P, H, P], BF, tag="x")
                for hf in range(2):
                    # z-bias matmuls accumulate into b_ps [q, kt*H + h]
                    b_ps = b_psp.tile([P, HKT * H], FP, tag="b")
                    for g in range(gph):
                        gidx = kc * gpc + hf * gph + g
                        z_sb = zpool.tile([P, tpg, NQC], BF, tag="z")
                        nc.sync.dma_start(z_sb, zT[gidx])
                        for t in range(tpg):
                            kt = g * tpg + t
                            nc.tensor.matmul(b_ps[:, kt * H:(kt + 1) * H],
                                             lhsT=z_sb[:, t, :], rhs=wb_sb,
                                             start=(kt == 0),
                                             stop=(kt == HKT - 1))
                    # batched add + exp for this half-chunk (all 8 heads)
                    e_sb = epool.tile([P, H, HKT], FP, tag="e")
                    nc.vector.tensor_tensor(
                        e_sb,
                        s_sb[:, :, kc * P + hf * HKT:kc * P + (hf + 1) * HKT],
                        b_ps.rearrange("p (kt h) -> p h kt", h=H), add)
                    nc.scalar.activation(x_sb[:, :, hf * HKT:(hf + 1) * HKT],
                                         e_sb,
                                         mybir.ActivationFunctionType.Exp)
                for hg in range(2):          # head groups of 4
                    t_ps = t_psp.tile([P, 4, P], BF, tag="t")
                    for hl in range(4):
                        nc.tensor.transpose(t_ps[:, hl, :],
                                            x_sb[:, hg * 4 + hl, :], ident_bf)
                    at_sb = atpool.tile([P, 4, P], BF, tag="at")
                    nc.vector.tensor_copy(at_sb, t_ps)
                    for hl in range(4):
                        h = hg * 4 + hl
                        # o_ps lives in one bank: open the accumulation group
                        # on the first matmul only, close on the last.
                        nc.tensor.matmul(
                            o_ps[:, h * (D + 1):(h + 1) * (D + 1)],
                            lhsT=at_sb[:, hl, :], rhs=vaug_sb[:, kc, h, :],
                            start=(kc == 0 and h == 0),
                            stop=(kc == kc_n - 1 and h == H - 1))

            # ---- epilogue: normalize, transpose, output projection ----
            recip_sb = const.tile([P, H], FP)
            for h in range(H):
                nc.vector.reciprocal(recip_sb[:, h:h + 1],
                                     o_ps[:, h * (D + 1) + D:h * (D + 1) + D + 1])
            o_sb = const.tile([P, 2, P], FP)     # [q, half, (h d)%128]
            for h in range(H):
                nc.vector.tensor_scalar(
                    o_sb[:, h // 4, (h % 4) * 32:(h % 4) * 32 + 32],
                    o_ps[:, h * (D + 1):h * (D + 1) + D],
                    recip_sb[:, h:h + 1], None, mult)
            oT_sb = const.tile([P, 2, P], FP)
            for m in range(2):
                t_full = proj_ps.tile([P, 512], FP, tag="proj", name="t_full")
                t_ps = t_full[:, :P]
                nc.tensor.transpose(t_ps, o_sb[:, m, :], ident)
                nc.vector.tensor_copy(oT_sb[:, m, :], t_ps)
            ps = proj_ps.tile([P, 512], FP, tag="proj")
            for m in range(2):
                nc.tensor.matmul(ps[:, :CQ], lhsT=oT_sb[:, m, :],
                                 rhs=wp_sb[:, m, :], start=(m == 0), stop=False)
            nc.tensor.matmul(ps[:, :CQ], lhsT=ones_row, rhs=bp_sb,
                             start=False, stop=True)
            y_sb = const.tile([P, CQ], FP)
            nc.vector.tensor_copy(y_sb, ps[:, :CQ])
            nc.sync.dma_start(y[:], y_sb)

    nc.compile()
    return nc


def prep_inputs(x_q, x_kv, z, Wq, bq, Wkv, bkv, Wb, bb, Wp, bp,
                nk=1024, gk=GK):
    """Host-side shard prep.  Returns in_maps for the 8 cores."""
    ng = nk // gk
    xkvT = np.ascontiguousarray(x_kv[0].T).astype(NP_BF)     # [CKV, nk]
    shared = dict(xkvT=xkvT,
                  Wq=np.ascontiguousarray(Wq).astype(NP_BF),
                  bq=np.ascontiguousarray(bq, dtype=np.float32),
                  Wkv=np.ascontiguousarray(Wkv).astype(NP_BF),
                  bkv=np.ascontiguousarray(bkv, dtype=np.float32),
                  Wb=np.ascontiguousarray(Wb).astype(NP_BF),
                  bb=np.ascontiguousarray(bb, dtype=np.float32),
                  Wp=np.ascontiguousarray(Wp, dtype=np.float32),
                  bp=np.ascontiguousarray(bp, dtype=np.float32))
    in_maps = []
    for i in range(NCORES):
        qs = i * NQC
        zi = z[0, qs:qs + NQC]                           # [q, k, c]
        # -> [g, c, t, q] with k = g*gk + t
        zi = zi.reshape(NQC, ng, gk, BD).transpose(1, 3, 2, 0)
        in_maps.append(dict(
            zT=np.ascontiguousarray(zi).astype(NP_BF),
            xqT=np.ascontiguousarray(x_q[0, qs:qs + NQC].T).astype(NP_BF),
            **shared,
        ))
    return in_maps


_NC_CACHE = {}


def kernel(x_q, x_kv, z, Wq, bq, Wkv, bkv, Wb, bb, Wp, bp):
    key = "full"
    if key not in _NC_CACHE:
        _NC_CACHE[key] = build_program()
    nc = _NC_CACHE[key]
    in_maps = prep_inputs(x_q, x_kv, z, Wq, bq, Wkv, bkv, Wb, bb, Wp, bp)
    res = run_bass_kernel_spmd(nc, in_maps, list(range(NCORES)))
    out = np.empty((1, NQ, CQ), dtype=np.float32)
    for i in range(NCORES):
        out[0, i * NQC:(i + 1) * NQC, :] = res.results[i]["y"]
    return out


# revision 20
# speedup vs baseline: 1.0350x; 1.0157x over previous
"""BiasAttention TRN2 kernel — q-sharded across 8 NeuronCores.

Each core owns a block of 128 queries and computes the full attention for
them (all 8 heads, all 1024 keys), including the z-bias projection, with no
collectives.  Host-side prep re-lays z out per core as [g, c, t, q] so the
bias-projection tiles arrive in SBUF matmul-ready (contract dim c on
partitions), and casts the matmul datapath to bf16 (TRN2 fp32 matmuls run
in split LOW/HIGH mode at ~4x the cost; accumulation stays fp32 in PSUM).
"""

import sys

if "/opt/trn_rl_repo" not in sys.path:
    sys.path.insert(0, "/opt/trn_rl_repo")

import ml_dtypes
import numpy as np

import concourse.bass as bass
import concourse.mybir as mybir
from concourse import bacc
from concourse.bass_utils import run_bass_kernel_spmd
from concourse.masks import make_identity
from concourse.tile import TileContext

P = 128          # partitions
H = 8            # heads
D = 32           # head dim
CQ = 256         # q channels
CKV = 256        # kv channels
BD = 128         # bias (z) channels
NQ = 1024        # total queries
NCORES = 8
NQC = NQ // NCORES   # queries per core = 128
SCALE = D ** (-0.5)

GK = 16          # z k-tiles per DMA group (shared host/device)
FP = mybir.dt.float32
BF = mybir.dt.bfloat16
NP_BF = ml_dtypes.bfloat16


def build_program(nk=1024, gk=GK):
    """Build the SPMD single-core program.  nk = number of keys, gk = z
    k-tiles per DMA group."""
    kc_n = nk // P            # k-chunks of 128
    ng = nk // gk             # z DMA groups
    add = mybir.AluOpType.add
    mult = mybir.AluOpType.mult

    nc = bacc.Bacc("TRN2", target_bir_lowering=False, debug=False,
                   num_devices=NCORES)

    # ---- I/O ----
    zT = nc.dram_tensor("zT", [ng, BD, gk, NQC], BF, kind="ExternalInput")
    xqT = nc.dram_tensor("xqT", [CQ, NQC], BF, kind="ExternalInput")
    xkvT = nc.dram_tensor("xkvT", [CKV, nk], BF, kind="ExternalInput")
    Wq = nc.dram_tensor("Wq", [CQ, H * D], BF, kind="ExternalInput")
    bq = nc.dram_tensor("bq", [H * D], FP, kind="ExternalInput")
    Wkv = nc.dram_tensor("Wkv", [CKV, 2 * H * D], BF, kind="ExternalInput")
    bkv = nc.dram_tensor("bkv", [2 * H * D], FP, kind="ExternalInput")
    Wb = nc.dram_tensor("Wb", [BD, H], BF, kind="ExternalInput")
    bb = nc.dram_tensor("bb", [H], FP, kind="ExternalInput")
    Wp = nc.dram_tensor("Wp", [H * D, CQ], FP, kind="ExternalInput")
    bp = nc.dram_tensor("bp", [CQ], FP, kind="ExternalInput")
    y = nc.dram_tensor("y", [NQC, CQ], FP, kind="ExternalOutput")

    with TileContext(nc) as tc:
        with (
            tc.tile_pool(name="const", bufs=1) as const,
            tc.tile_pool(name="zpool", bufs=16) as zpool,
            tc.tile_pool(name="epool", bufs=3) as epool,
            tc.tile_pool(name="atpool", bufs=3) as atpool,
            tc.tile_pool(name="proj_ps", bufs=3, space="PSUM") as proj_ps,
            tc.tile_pool(name="b_ps", bufs=2, space="PSUM") as b_psp,
            tc.tile_pool(name="t_ps", bufs=2, space="PSUM") as t_psp,
            tc.tile_pool(name="o_ps", bufs=1, space="PSUM") as o_psp,
        ):
            # ---- z stream head-start: wb + first z groups lead the ring ----
            wb_sb = const.tile([P, H], BF)
            nc.sync.dma_start(wb_sb, Wb[:])
            zpre = []
            for gidx in range(4):
                z_sb = zpool.tile([P, 16, NQC], BF, tag="z", name=f"zpre{gidx}")
                nc.sync.dma_start(z_sb, zT[gidx])
                zpre.append(z_sb)

            # ---- constants / weights to SBUF ----
            wq_sb = const.tile([P, 2, H * D], BF)
            nc.sync.dma_start(wq_sb, Wq.rearrange("(o p) m -> p o m", p=P))
            wkv_sb = const.tile([P, 2, 2 * H * D], BF)
            nc.sync.dma_start(wkv_sb, Wkv.rearrange("(o p) m -> p o m", p=P))
            wp_sb = const.tile([P, 2, CQ], FP)
            nc.sync.dma_start(wp_sb, Wp.rearrange("(o p) m -> p o m", p=P))
            xqT_sb = const.tile([P, 2, NQC], BF)
            nc.sync.dma_start(xqT_sb, xqT.rearrange("(o p) q -> p o q", p=P))
            xkvT_sb = const.tile([P, 2, nk], BF)
            nc.sync.dma_start(xkvT_sb, xkvT.rearrange("(o p) k -> p o k", p=P))
            bq_sb = const.tile([P, 2], FP)
            nc.sync.dma_start(bq_sb, bq.rearrange("(o p) -> p o", p=P))
            bkvK_sb = const.tile([P, 2], FP)
            nc.sync.dma_start(bkvK_sb, bkv[0:H * D].rearrange("(o p) -> p o", p=P))
            bkvV_sb = const.tile([1, H * D], FP)
            nc.sync.dma_start(bkvV_sb, bkv[None, H * D:2 * H * D])
            bp_sb = const.tile([1, CQ], FP)
            nc.sync.dma_start(bp_sb, bp[None, :])
            bb_ap = bb[:]
            bb_sb = const.tile([P, H], FP)
            nc.gpsimd.dma_start(
                out=bb_sb,
                in_=bass.AP(tensor=bb_ap.tensor, offset=bb_ap.offset,
                            ap=[[0, P]] + list(bb_ap.ap)),
            )
            ident = const.tile([P, P], FP)
            make_identity(nc, ident)
            ident_bf = const.tile([P, P], BF)
            make_identity(nc, ident_bf)
            ones_row = const.tile([1, P], FP)
            nc.vector.memset(ones_row, 1.0)

            # V augmented with a ones column per head: [k, kc, h, D+1]
            vaug_sb = const.tile([P, kc_n, H, D + 1], BF)
            nc.vector.memset(vaug_sb, 1.0)


            # ---- projections (bf16 in, fp32 psum accumulate) ----
            # Q^T [(h d), q] with (x + bq) * SCALE folded in, stored bf16
            qT_sb = const.tile([P, 2, NQC], BF)
            for m in range(2):
                ps = proj_ps.tile([P, 512], FP, tag="proj")
                for c in range(2):
                    nc.tensor.matmul(ps[:, :NQC],
                                     lhsT=wq_sb[:, c, m * P:(m + 1) * P],
                                     rhs=xqT_sb[:, c, :],
                                     start=(c == 0), stop=(c == 1))
                nc.vector.tensor_scalar(qT_sb[:, m, :], ps[:, :NQC],
                                        bq_sb[:, m:m + 1], SCALE, add, mult)

            # K^T [(h d), k] with +bkv_K, stored bf16
            kT_sb = const.tile([P, 2, nk], BF)
            for m in range(2):
                for nh in range((nk + 511) // 512):
                    nn_ = min(512, nk - nh * 512)
                    ps = proj_ps.tile([P, 512], FP, tag="proj")
                    for c in range(2):
                        nc.tensor.matmul(ps[:, :nn_],
                                         lhsT=wkv_sb[:, c, m * P:(m + 1) * P],
                                         rhs=xkvT_sb[:, c, nh * 512:nh * 512 + nn_],
                                         start=(c == 0), stop=(c == 1))
                    nc.vector.tensor_scalar(kT_sb[:, m, nh * 512:nh * 512 + nn_],
                                            ps[:, :nn_], bkvK_sb[:, m:m + 1],
                                            None, add)


            # S[q, h, k] = SCALE * Q K^T + bb  (scale folded into Q already)
            s_sb = const.tile([P, H, nk], FP)
            for h in range(H):
                r0 = (h % 4) * 32
                for nh in range((nk + 511) // 512):
                    nn_ = min(512, nk - nh * 512)
                    ps = proj_ps.tile([P, 512], FP, tag="proj", name="qk_ps")
                    nc.tensor.matmul(ps[:, :nn_],
                                     lhsT=qT_sb[r0:r0 + 32, h // 4, :],
                                     rhs=kT_sb[r0:r0 + 32, h // 4,
                                               nh * 512:nh * 512 + nn_],
                                     start=True, stop=True,
                                     tile_position=(r0, 0))
                    # Identity-with-bias folds bb[h] into S; alternate the
                    # copy between ACT and DVE to halve the serial chain.
                    if (h * 2 + nh) % 2 == 0:
                        nc.scalar.activation(
                            s_sb[:, h, nh * 512:nh * 512 + nn_], ps[:, :nn_],
                            mybir.ActivationFunctionType.Identity,
                            bias=bb_sb[:, h:h + 1])
                    else:
                        nc.vector.tensor_scalar(
                            s_sb[:, h, nh * 512:nh * 512 + nn_], ps[:, :nn_],
                            bb_sb[:, h:h + 1], None, add)



            # V [k, (h d)] + bkv_V, written into vaug (ones col preserved);
            # emitted after QK so the S chain starts first.
            for kc in range(kc_n):
                ps = proj_ps.tile([P, 512], FP, tag="proj", name="v_ps")
                for c in range(2):
                    nc.tensor.matmul(ps[:, :H * D],
                                     lhsT=xkvT_sb[:, c, kc * P:(kc + 1) * P],
                                     rhs=wkv_sb[:, c, H * D:2 * H * D],
                                     start=(c == 0), stop=False)
                nc.tensor.matmul(ps[:, :H * D], lhsT=ones_row,
                                 rhs=bkvV_sb, start=False, stop=True)
                nc.scalar.activation(
                    vaug_sb[:, kc, :, 0:D],
                    ps[:, :H * D].rearrange("p (h d) -> p h d", h=H),
                    mybir.ActivationFunctionType.Copy)
            # ---- main loop over k-chunks ----
            o_ps = o_psp.tile([P, H * (D + 1)], FP)   # [q, h*(D+1)]
            gpc = P // gk if gk < P else 1      # groups per k-chunk
            tpg = min(gk, P)                    # k-tiles per group
            HKT = 64                      # k-tiles per half-chunk (1 psum bank)
            gph = HKT // tpg              # z DMA groups per half-chunk
            for kc in range(kc_n):
                x_sb = epool.tile([P, H, P], BF, tag="x")
                for hf in range(2):
                    # z-bias matmuls accumulate into b_ps [q, kt*H + h]
                    b_ps = b_psp.tile([P, HKT * H], FP, tag="b")
                    for g in range(gph):
                        gidx = kc * gpc + hf * gph + g
                        if gidx < len(zpre):
                            z_sb = zpre[gidx]
                        else:
                            z_sb = zpool.tile([P, tpg, NQC], BF, tag="z")
                            nc.sync.dma_start(z_sb, zT[gidx])
                        for t in range(tpg):
                            kt = g * tpg + t
                            nc.tensor.matmul(b_ps[:, kt * H:(kt + 1) * H],
                                             lhsT=z_sb[:, t, :], rhs=wb_sb,
                                             start=(kt == 0),
                                             stop=(kt == HKT - 1))
                    # batched add + exp for this half-chunk (all 8 heads)
                    e_sb = epool.tile([P, H, HKT], FP, tag="e")
                    nc.vector.tensor_tensor(
                        e_sb,
                        s_sb[:, :, kc * P + hf * HKT:kc * P + (hf + 1) * HKT],
                        b_ps.rearrange("p (kt h) -> p h kt", h=H), add)
                    nc.scalar.activation(x_sb[:, :, hf * HKT:(hf + 1) * HKT],
                                         e_sb,
                                         mybir.ActivationFunctionType.Exp)
                for hg in range(2):          # head groups of 4
                    t_ps = t_psp.tile([P, 4, P], BF, tag="t")
                    for hl in range(4):
                        nc.tensor.transpose(t_ps[:, hl, :],
                                            x_sb[:, hg * 4 + hl, :], ident_bf)
                    at_sb = atpool.tile([P, 4, P], BF, tag="at")
                    nc.vector.tensor_copy(at_sb, t_ps)
                    for hl in range(4):
                        h = hg * 4 + hl
                        # o_ps lives in one bank: open the accumulation group
                        # on the first matmul only, close on the last.
                        nc.tensor.matmul(
                            o_ps[:, h * (D + 1):(h + 1) * (D + 1)],
                            lhsT=at_sb[:, hl, :], rhs=vaug_sb[:, kc, h, :],
                            start=(kc == 0 and h == 0),
                            stop=(kc == kc_n - 1 and h == H - 1))

            # ---- epilogue: normalize, transpose, output projection ----
            recip_sb = const.tile([P, H], FP)
            for h in range(H):
                nc.vector.reciprocal(recip_sb[:, h:h + 1],
                                     o_ps[:, h * (D + 1) + D:h * (D + 1) + D + 1])
            o_sb = const.tile([P, 2, P], FP)     # [q, half, (h d)%128]
            for h in range(H):
                nc.vector.tensor_scalar(
                    o_sb[:, h // 4, (h % 4) * 32:(h % 4) * 32 + 32],
                    o_ps[:, h * (D + 1):h * (D + 1) + D],
                    recip_sb[:, h:h + 1], None, mult)
            oT_sb = const.tile([P, 2, P], FP)
            for m in range(2):
                t_full = proj_ps.tile([P, 512], FP, tag="proj", name="t_full")
                t_ps = t_full[:, :P]
                nc.tensor.transpose(t_ps, o_sb[:, m, :], ident)
                nc.vector.tensor_copy(oT_sb[:, m, :], t_ps)
            ps = proj_ps.tile([P, 512], FP, tag="proj")
            for m in range(2):
                nc.tensor.matmul(ps[:, :CQ], lhsT=oT_sb[:, m, :],
                                 rhs=wp_sb[:, m, :], start=(m == 0), stop=False)
            nc.tensor.matmul(ps[:, :CQ], lhsT=ones_row, rhs=bp_sb,
                             start=False, stop=True)
            y_sb = const.tile([P, CQ], FP)
            nc.vector.tensor_copy(y_sb, ps[:, :CQ])
            nc.sync.dma_start(y[:], y_sb)

    nc.compile()
    return nc


def prep_inputs(x_q, x_kv, z, Wq, bq, Wkv, bkv, Wb, bb, Wp, bp,
                nk=1024, gk=GK):
    """Host-side shard prep.  Returns in_maps for the 8 cores."""
    ng = nk // gk
    xkvT = np.ascontiguousarray(x_kv[0].T).astype(NP_BF)     # [CKV, nk]
    shared = dict(xkvT=xkvT,
                  Wq=np.ascontiguousarray(Wq).astype(NP_BF),
                  bq=np.ascontiguousarray(bq, dtype=np.float32),
                  Wkv=np.ascontiguousarray(Wkv).astype(NP_BF),
                  bkv=np.ascontiguousarray(bkv, dtype=np.float32),
                  Wb=np.ascontiguousarray(Wb).astype(NP_BF),
                  bb=np.ascontiguousarray(bb, dtype=np.float32),
                  Wp=np.ascontiguousarray(Wp, dtype=np.float32),
                  bp=np.ascontiguousarray(bp, dtype=np.float32))
    in_maps = []
    for i in range(NCORES):
        qs = i * NQC
        zi = z[0, qs:qs + NQC]                           # [q, k, c]
        # -> [g, c, t, q] with k = g*gk + t
        zi = zi.reshape(NQC, ng, gk, BD).transpose(1, 3, 2, 0)
        in_maps.append(dict(
            zT=np.ascontiguousarray(zi).astype(NP_BF),
            xqT=np.ascontiguousarray(x_q[0, qs:qs + NQC].T).astype(NP_BF),
            **shared,
        ))
    return in_maps


_NC_CACHE = {}


def kernel(x_q, x_kv, z, Wq, bq, Wkv, bkv, Wb, bb, Wp, bp):
    key = "full"
    if key not in _NC_CACHE:
        _NC_CACHE[key] = build_program()
    nc = _NC_CACHE[key]
    in_maps = prep_inputs(x_q, x_kv, z, Wq, bq, Wkv, bkv, Wb, bb, Wp, bp)
    res = run_bass_kernel_spmd(nc, in_maps, list(range(NCORES)))
    out = np.empty((1, NQ, CQ), dtype=np.float32)
    for i in range(NCORES):
        out[0, i * NQC:(i + 1) * NQC, :] = res.results[i]["y"]
    return out


# revision 21
# speedup vs baseline: 1.0545x; 1.0189x over previous
"""BiasAttention TRN2 kernel — q-sharded across 8 NeuronCores.

Each core owns a block of 128 queries and computes the full attention for
them (all 8 heads, all 1024 keys), including the z-bias projection, with no
collectives.  Host-side prep re-lays z out per core as [g, c, t, q] so the
bias-projection tiles arrive in SBUF matmul-ready (contract dim c on
partitions), and casts the matmul datapath to bf16 (TRN2 fp32 matmuls run
in split LOW/HIGH mode at ~4x the cost; accumulation stays fp32 in PSUM).
"""

import sys

if "/opt/trn_rl_repo" not in sys.path:
    sys.path.insert(0, "/opt/trn_rl_repo")

import ml_dtypes
import numpy as np

import concourse.bass as bass
import concourse.mybir as mybir
from concourse import bacc
from concourse.bass_utils import run_bass_kernel_spmd
from concourse.masks import make_identity
from concourse.tile import TileContext

P = 128          # partitions
H = 8            # heads
D = 32           # head dim
CQ = 256         # q channels
CKV = 256        # kv channels
BD = 128         # bias (z) channels
NQ = 1024        # total queries
NCORES = 8
NQC = NQ // NCORES   # queries per core = 128
SCALE = D ** (-0.5)

GK = 16          # z k-tiles per DMA group (shared host/device)
FP = mybir.dt.float32
BF = mybir.dt.bfloat16
NP_BF = ml_dtypes.bfloat16


def build_program(nk=1024, gk=GK):
    """Build the SPMD single-core program.  nk = number of keys, gk = z
    k-tiles per DMA group."""
    kc_n = nk // P            # k-chunks of 128
    ng = nk // gk             # z DMA groups
    add = mybir.AluOpType.add
    mult = mybir.AluOpType.mult

    nc = bacc.Bacc("TRN2", target_bir_lowering=False, debug=False,
                   num_devices=NCORES)

    # ---- I/O ----
    zT = nc.dram_tensor("zT", [ng, BD, gk, NQC], BF, kind="ExternalInput")
    xqT = nc.dram_tensor("xqT", [CQ, NQC], BF, kind="ExternalInput")
    xkvT = nc.dram_tensor("xkvT", [CKV, nk], BF, kind="ExternalInput")
    Wq = nc.dram_tensor("Wq", [CQ, H * D], BF, kind="ExternalInput")
    bq = nc.dram_tensor("bq", [H * D], FP, kind="ExternalInput")
    Wkv = nc.dram_tensor("Wkv", [CKV, 2 * H * D], BF, kind="ExternalInput")
    bkv = nc.dram_tensor("bkv", [2 * H * D], FP, kind="ExternalInput")
    Wb = nc.dram_tensor("Wb", [BD, H], BF, kind="ExternalInput")
    bb = nc.dram_tensor("bb", [H], FP, kind="ExternalInput")
    Wp = nc.dram_tensor("Wp", [H * D, CQ], FP, kind="ExternalInput")
    bp = nc.dram_tensor("bp", [CQ], FP, kind="ExternalInput")
    y = nc.dram_tensor("y", [NQC, CQ], FP, kind="ExternalOutput")

    with TileContext(nc) as tc:
        with (
            tc.tile_pool(name="const", bufs=1) as const,
            tc.tile_pool(name="zpool", bufs=20) as zpool,
            tc.tile_pool(name="epool", bufs=3) as epool,
            tc.tile_pool(name="atpool", bufs=3) as atpool,
            tc.tile_pool(name="proj_ps", bufs=2, space="PSUM") as proj_ps,
            tc.tile_pool(name="b_ps", bufs=3, space="PSUM") as b_psp,
            tc.tile_pool(name="t_ps", bufs=2, space="PSUM") as t_psp,
            tc.tile_pool(name="o_ps", bufs=1, space="PSUM") as o_psp,
        ):
            # ---- z stream head-start: wb + first z groups lead the ring ----
            wb_sb = const.tile([P, H], BF)
            nc.sync.dma_start(wb_sb, Wb[:])
            zpre = []
            for gidx in range(4):
                z_sb = zpool.tile([P, 16, NQC], BF, tag="z", name=f"zpre{gidx}")
                nc.sync.dma_start(z_sb, zT[gidx])
                zpre.append(z_sb)

            # ---- constants / weights to SBUF ----
            wq_sb = const.tile([P, 2, H * D], BF)
            nc.sync.dma_start(wq_sb, Wq.rearrange("(o p) m -> p o m", p=P))
            wkv_sb = const.tile([P, 2, 2 * H * D], BF)
            nc.sync.dma_start(wkv_sb, Wkv.rearrange("(o p) m -> p o m", p=P))
            wp_sb = const.tile([P, 2, CQ], FP)
            nc.sync.dma_start(wp_sb, Wp.rearrange("(o p) m -> p o m", p=P))
            xqT_sb = const.tile([P, 2, NQC], BF)
            nc.sync.dma_start(xqT_sb, xqT.rearrange("(o p) q -> p o q", p=P))
            xkvT_sb = const.tile([P, 2, nk], BF)
            nc.sync.dma_start(xkvT_sb, xkvT.rearrange("(o p) k -> p o k", p=P))
            bq_sb = const.tile([P, 2], FP)
            nc.sync.dma_start(bq_sb, bq.rearrange("(o p) -> p o", p=P))
            bkvK_sb = const.tile([P, 2], FP)
            nc.sync.dma_start(bkvK_sb, bkv[0:H * D].rearrange("(o p) -> p o", p=P))
            bkvV_sb = const.tile([1, H * D], FP)
            nc.sync.dma_start(bkvV_sb, bkv[None, H * D:2 * H * D])
            bp_sb = const.tile([1, CQ], FP)
            nc.sync.dma_start(bp_sb, bp[None, :])
            bb_ap = bb[:]
            bb_sb = const.tile([P, H], FP)
            nc.gpsimd.dma_start(
                out=bb_sb,
                in_=bass.AP(tensor=bb_ap.tensor, offset=bb_ap.offset,
                            ap=[[0, P]] + list(bb_ap.ap)),
            )
            ident = const.tile([P, P], FP)
            make_identity(nc, ident)
            ident_bf = const.tile([P, P], BF)
            make_identity(nc, ident_bf)
            ones_row = const.tile([1, P], FP)
            nc.vector.memset(ones_row, 1.0)

            # V augmented with a ones column per head: [k, kc, h, D+1]
            vaug_sb = const.tile([P, kc_n, H, D + 1], BF)
            nc.vector.memset(vaug_sb, 1.0)


            # ---- projections (bf16 in, fp32 psum accumulate) ----
            # Q^T [(h d), q] with (x + bq) * SCALE folded in, stored bf16
            qT_sb = const.tile([P, 2, NQC], BF)
            for m in range(2):
                ps = proj_ps.tile([P, 512], FP, tag="proj")
                for c in range(2):
                    nc.tensor.matmul(ps[:, :NQC],
                                     lhsT=wq_sb[:, c, m * P:(m + 1) * P],
                                     rhs=xqT_sb[:, c, :],
                                     start=(c == 0), stop=(c == 1))
                nc.vector.tensor_scalar(qT_sb[:, m, :], ps[:, :NQC],
                                        bq_sb[:, m:m + 1], SCALE, add, mult)

            # K^T [(h d), k] with +bkv_K, stored bf16
            kT_sb = const.tile([P, 2, nk], BF)
            for m in range(2):
                for nh in range((nk + 511) // 512):
                    nn_ = min(512, nk - nh * 512)
                    ps = proj_ps.tile([P, 512], FP, tag="proj")
                    for c in range(2):
                        nc.tensor.matmul(ps[:, :nn_],
                                         lhsT=wkv_sb[:, c, m * P:(m + 1) * P],
                                         rhs=xkvT_sb[:, c, nh * 512:nh * 512 + nn_],
                                         start=(c == 0), stop=(c == 1))
                    nc.vector.tensor_scalar(kT_sb[:, m, nh * 512:nh * 512 + nn_],
                                            ps[:, :nn_], bkvK_sb[:, m:m + 1],
                                            None, add)


            # S[q, h, k] = SCALE * Q K^T + bb  (scale folded into Q already)
            s_sb = const.tile([P, H, nk], FP)
            for h in range(H):
                r0 = (h % 4) * 32
                for nh in range((nk + 511) // 512):
                    nn_ = min(512, nk - nh * 512)
                    ps = proj_ps.tile([P, 512], FP, tag="proj", name="qk_ps")
                    nc.tensor.matmul(ps[:, :nn_],
                                     lhsT=qT_sb[r0:r0 + 32, h // 4, :],
                                     rhs=kT_sb[r0:r0 + 32, h // 4,
                                               nh * 512:nh * 512 + nn_],
                                     start=True, stop=True,
                                     tile_position=(r0, 0))
                    # Identity-with-bias folds bb[h] into S; alternate the
                    # copy between ACT and DVE to halve the serial chain.
                    if (h * 2 + nh) % 2 == 0:
                        nc.scalar.activation(
                            s_sb[:, h, nh * 512:nh * 512 + nn_], ps[:, :nn_],
                            mybir.ActivationFunctionType.Identity,
                            bias=bb_sb[:, h:h + 1])
                    else:
                        nc.vector.tensor_scalar(
                            s_sb[:, h, nh * 512:nh * 512 + nn_], ps[:, :nn_],
                            bb_sb[:, h:h + 1], None, add)



            # V [k, (h d)] + bkv_V, written into vaug (ones col preserved);
            # emitted after QK so the S chain starts first.
            for kc in range(kc_n):
                ps = proj_ps.tile([P, 512], FP, tag="proj", name="v_ps")
                for c in range(2):
                    nc.tensor.matmul(ps[:, :H * D],
                                     lhsT=xkvT_sb[:, c, kc * P:(kc + 1) * P],
                                     rhs=wkv_sb[:, c, H * D:2 * H * D],
                                     start=(c == 0), stop=False)
                nc.tensor.matmul(ps[:, :H * D], lhsT=ones_row,
                                 rhs=bkvV_sb, start=False, stop=True)
                nc.scalar.activation(
                    vaug_sb[:, kc, :, 0:D],
                    ps[:, :H * D].rearrange("p (h d) -> p h d", h=H),
                    mybir.ActivationFunctionType.Copy)
            # ---- main loop over k-chunks ----
            o_ps = o_psp.tile([P, H * (D + 1)], FP)   # [q, h*(D+1)]
            gpc = P // gk if gk < P else 1      # groups per k-chunk
            tpg = min(gk, P)                    # k-tiles per group
            HKT = 64                      # k-tiles per half-chunk (1 psum bank)
            gph = HKT // tpg              # z DMA groups per half-chunk
            for kc in range(kc_n):
                x_sb = epool.tile([P, H, P], BF, tag="x")
                for hf in range(2):
                    # z-bias matmuls accumulate into b_ps [q, kt*H + h]
                    b_ps = b_psp.tile([P, HKT * H], FP, tag="b")
                    for g in range(gph):
                        gidx = kc * gpc + hf * gph + g
                        if gidx < len(zpre):
                            z_sb = zpre[gidx]
                        else:
                            z_sb = zpool.tile([P, tpg, NQC], BF, tag="z")
                            nc.sync.dma_start(z_sb, zT[gidx])
                        for t in range(tpg):
                            kt = g * tpg + t
                            nc.tensor.matmul(b_ps[:, kt * H:(kt + 1) * H],
                                             lhsT=z_sb[:, t, :], rhs=wb_sb,
                                             start=(kt == 0),
                                             stop=(kt == HKT - 1))
                    # batched add + exp for this half-chunk (all 8 heads)
                    e_sb = epool.tile([P, H, HKT], FP, tag="e")
                    nc.vector.tensor_tensor(
                        e_sb,
                        s_sb[:, :, kc * P + hf * HKT:kc * P + (hf + 1) * HKT],
                        b_ps.rearrange("p (kt h) -> p h kt", h=H), add)
                    nc.scalar.activation(x_sb[:, :, hf * HKT:(hf + 1) * HKT],
                                         e_sb,
                                         mybir.ActivationFunctionType.Exp)
                for hg in range(2):          # head groups of 4
                    t_ps = t_psp.tile([P, 4, P], BF, tag="t")
                    for hl in range(4):
                        nc.tensor.transpose(t_ps[:, hl, :],
                                            x_sb[:, hg * 4 + hl, :], ident_bf)
                    at_sb = atpool.tile([P, 4, P], BF, tag="at")
                    nc.vector.tensor_copy(at_sb, t_ps)
                    for hl in range(4):
                        h = hg * 4 + hl
                        # o_ps lives in one bank: open the accumulation group
                        # on the first matmul only, close on the last.
                        nc.tensor.matmul(
                            o_ps[:, h * (D + 1):(h + 1) * (D + 1)],
                            lhsT=at_sb[:, hl, :], rhs=vaug_sb[:, kc, h, :],
                            start=(kc == 0 and h == 0),
                            stop=(kc == kc_n - 1 and h == H - 1))

            # ---- epilogue: normalize, transpose, output projection ----
            recip_sb = const.tile([P, H], FP)
            for h in range(H):
                nc.vector.reciprocal(recip_sb[:, h:h + 1],
                                     o_ps[:, h * (D + 1) + D:h * (D + 1) + D + 1])
            o_sb = const.tile([P, 2, P], FP)     # [q, half, (h d)%128]
            for h in range(H):
                nc.vector.tensor_scalar(
                    o_sb[:, h // 4, (h % 4) * 32:(h % 4) * 32 + 32],
                    o_ps[:, h * (D + 1):h * (D + 1) + D],
                    recip_sb[:, h:h + 1], None, mult)
            oT_sb = const.tile([P, 2, P], FP)
            for m in range(2):
                t_full = proj_ps.tile([P, 512], FP, tag="proj", name="t_full")
                t_ps = t_full[:, :P]
                nc.tensor.transpose(t_ps, o_sb[:, m, :], ident)
                nc.vector.tensor_copy(oT_sb[:, m, :], t_ps)
            ps = proj_ps.tile([P, 512], FP, tag="proj")
            for m in range(2):
                nc.tensor.matmul(ps[:, :CQ], lhsT=oT_sb[:, m, :],
                                 rhs=wp_sb[:, m, :], start=(m == 0), stop=False)
            nc.tensor.matmul(ps[:, :CQ], lhsT=ones_row, rhs=bp_sb,
                             start=False, stop=True)
            y_sb = const.tile([P, CQ], FP)
            nc.vector.tensor_copy(y_sb, ps[:, :CQ])
            nc.sync.dma_start(y[:], y_sb)

    nc.compile()
    return nc


def prep_inputs(x_q, x_kv, z, Wq, bq, Wkv, bkv, Wb, bb, Wp, bp,
                nk=1024, gk=GK):
    """Host-side shard prep.  Returns in_maps for the 8 cores."""
    ng = nk // gk
    xkvT = np.ascontiguousarray(x_kv[0].T).astype(NP_BF)     # [CKV, nk]
    shared = dict(xkvT=xkvT,
                  Wq=np.ascontiguousarray(Wq).astype(NP_BF),
                  bq=np.ascontiguousarray(bq, dtype=np.float32),
                  Wkv=np.ascontiguousarray(Wkv).astype(NP_BF),
                  bkv=np.ascontiguousarray(bkv, dtype=np.float32),
                  Wb=np.ascontiguousarray(Wb).astype(NP_BF),
                  bb=np.ascontiguousarray(bb, dtype=np.float32),
                  Wp=np.ascontiguousarray(Wp, dtype=np.float32),
                  bp=np.ascontiguousarray(bp, dtype=np.float32))
    in_maps = []
    for i in range(NCORES):
        qs = i * NQC
        zi = z[0, qs:qs + NQC]                           # [q, k, c]
        # -> [g, c, t, q] with k = g*gk + t
        zi = zi.reshape(NQC, ng, gk, BD).transpose(1, 3, 2, 0)
        in_maps.append(dict(
            zT=np.ascontiguousarray(zi).astype(NP_BF),
            xqT=np.ascontiguousarray(x_q[0, qs:qs + NQC].T).astype(NP_BF),
            **shared,
        ))
    return in_maps


_NC_CACHE = {}


def kernel(x_q, x_kv, z, Wq, bq, Wkv, bkv, Wb, bb, Wp, bp):
    key = "full"
    if key not in _NC_CACHE:
        _NC_CACHE[key] = build_program()
    nc = _NC_CACHE[key]
    in_maps = prep_inputs(x_q, x_kv, z, Wq, bq, Wkv, bkv, Wb, bb, Wp, bp)
    res = run_bass_kernel_spmd(nc, in_maps, list(range(NCORES)))
    out = np.empty((1, NQ, CQ), dtype=np.float32)
    for i in range(NCORES):
        out[0, i * NQC:(i + 1) * NQC, :] = res.results[i]["y"]
    return out
